# revision 16
# baseline (speedup 1.0000x reference)
"""Trainium2 Bass kernel for nn_AttentionBlock (Set-Transformer MAB block).

Reference computation (per batch b):
    Qp = Q @ Wq.T + bq ; Kp = K @ Wk.T + bk ; Vp = K @ Wv.T + bv   (4 heads of 64)
    A  = softmax(Qp Kp^T / 8)  ;  ctx = A Vp
    O  = LN0(Qp + ctx) ;  O = O + relu(O @ Wo.T + bo) ;  out = LN1(O)

Sharding: data-parallel over (batch, query-half) -> 8 independent shards,
one per NeuronCore, no collectives.  Each core sees its 1024 queries, the
full 2048 keys of its batch, and all weights, shipped feature-major bf16.

v2 design notes (changes vs the first working kernel):
  * ACT exp is the pacing resource: 64 x [128,1024] exps ~ 71us.  Target
    is exp-stream density: startup to first exp minimized, no ACT table
    switches (we monkeypatch the act-table list so the single
    natural_log_exp_and_others set serves Exp/Ln/Identity/Relu/Copy),
    and the tail overlaps the last heads' merges.
  * DMA: inputs shipped as [128, N] partition-major tensors with 2-8KB
    contiguous rows; critical tensors (Wqkv, QT, KT chunk0) issued first
    on the two HWDGE queues.  PE warm-up dummies run during the DMA wait
    so the HAM clock-gate lifts (1.2 -> 2.4GHz) before the projections.
  * tail math: LayerNorm is invariant to per-token affine maps and relu
    is positively homogeneous, so with m/sd from bn_stats:
        z   = O + sd*relu(rs*(O@Wo^T - m*wsum + sd*bo)) = O + relu(p4)
        out = (z - m1) / sd1
    where p4 rides the Wo matmul as one K=2 correction matmul with
    stationary rows [m; sd] against [-wsum; bo].  sqrt is computed as
    exp(0.5*ln(v+eps)) on ACT (same table set as the softmax exps), and
    1/sd1 as exp(-0.5*ln(v1+eps)) -- no Sqrt table switch, no vector
    reciprocal on the critical path.
  * O is kept bf16 (transposes are 1-pass on the PE); the z residual is
    fp32.  Tail work is split ACT/Vector/PE to pipeline ~1.4us/query-tile.
"""

from contextlib import ExitStack

import ml_dtypes
import numpy as np

import concourse.bass as bass
import concourse.tile as tile
from concourse import bacc, mybir
from concourse.bass_utils import run_bass_kernel_spmd
from concourse.masks import make_identity

FP = mybir.dt.float32
BF = mybir.dt.bfloat16
AF = mybir.ActivationFunctionType
OP = mybir.AluOpType

B = 4
SQ_FULL = 2048   # queries per batch
SK = 2048        # keys per batch
D = 256
H = 4
DH = D // H      # 64
NCORES = 8
QSPLIT = 2
SQ = SQ_FULL // QSPLIT    # queries per core
NQT = SQ // 128           # 8 query tiles
NKT = SK // 128           # 16 key tiles
NDT = D // 128            # 2 feature tiles
QN = SQ // 512            # 2 query column blocks
KC = SK // 512            # 4 key column chunks
LN_EPS = 1e-5
SCALE = 0.125             # 1 / sqrt(DH)

# ---- ACT table-set pinning -------------------------------------------------
# The greedy act-table pass picks the FIRST set containing each activation
# function (Exp -> exp_and_others, Ln -> natural_log, ...), which would
# thrash ACT_TABLE_LOADs (~2.7us each).  Strip our functions from every set
# before natural_log_exp_and_others so that single set (which contains exp,
# ln, identity, relu, copy, memset_zero) serves the whole kernel: exactly
# one table load, issued before the first DMA completes.
_PINNED_SET = "natural_log_exp_and_others"
_PIN_FNS = {AF.Exp, AF.Ln, AF.Identity, AF.Relu, AF.Copy}
_tables_patched = False


def _patch_act_tables():
    global _tables_patched
    if _tables_patched:
        return
    import concourse.bacc as bacc_mod
    import concourse.hw_specs as hw_specs

    orig = hw_specs.get_activation_tables

    def patched(module_arch):
        tabs = orig(module_arch)
        if _PINNED_SET not in tabs or not (_PIN_FNS <= tabs[_PINNED_SET]):
            return tabs  # unexpected table layout: leave untouched
        out = {}
        seen_pinned = False
        for name, fns in tabs.items():
            if name == _PINNED_SET:
                seen_pinned = True
                out[name] = fns
            elif not seen_pinned:
                out[name] = fns - _PIN_FNS
            else:
                out[name] = fns
        return out

    bacc_mod.get_activation_tables = patched
    _tables_patched = True


def _emit(nc, skip_gb):
    # DRAM parameters: everything partition-major with fat contiguous rows.
    QTd = nc.declare_dram_parameter("QT", [128, QN * NDT * 512], BF, isOutput=False)
    KTd = nc.declare_dram_parameter("KT", [128, KC * NDT * 512], BF, isOutput=False)
    Wd = nc.declare_dram_parameter("Wall", [128, 4 * NDT * D], BF, isOutput=False)
    V1 = {
        n: nc.declare_dram_parameter(n, [D], FP, isOutput=False)
        for n in ("bq", "bk", "bv", "bo", "g0", "beta0", "g1", "beta1")
    }
    out = nc.declare_dram_parameter("out", [SQ, D], BF, isOutput=True)

    with tile.TileContext(nc) as tc, ExitStack() as ctx:
        singles = ctx.enter_context(tc.tile_pool(name="singles", bufs=1))
        big = ctx.enter_context(tc.tile_pool(name="big", bufs=1))
        ex = ctx.enter_context(tc.tile_pool(name="ex", bufs=4))
        ctp = ctx.enter_context(tc.tile_pool(name="ctp", bufs=2))
        outp = ctx.enter_context(tc.tile_pool(name="outp", bufs=8))
        ztp = ctx.enter_context(tc.tile_pool(name="ztp", bufs=4))

        QpT = big.tile([128, NDT, SQ], BF)
        KpT = big.tile([128, NDT, SK], BF)
        Vp = big.tile([128, NKT, H, DH + 1], BF)
        O = big.tile([128, NQT, D], BF)
        OT = big.tile([128, NDT, SQ], BF)
        recips = big.tile([128, NQT, H], FP)
        KT = big.tile([128, KC, NDT, 512], BF)
        QT = big.tile([128, QN, NDT, 512], BF)
        WT = big.tile([128, 4, NDT, D], BF)     # Wq | Wk | Wv | Wo
        # tail stats
        msd = big.tile([128, NQT, 2], FP, tag="msd")    # [m0, v0->sd0]
        mv1 = big.tile([128, NQT, 2], FP, tag="mv1")
        rs1 = big.tile([128, NQT], FP, tag="rs1")
        lnt = big.tile([128, NQT], FP, tag="lnt")
        st4 = big.tile([128, NQT, H, 6], FP, tag="st4")
        bgrow = big.tile([2, NQT, 128], BF, tag="bgrow")
        wb2 = singles.tile([2, D], BF, tag="wb2")
        negones = singles.tile([128, 1], BF, tag="negones")
        warm = singles.tile([128, 512], BF, tag="warm")  # garbage PE warm-up src

        ident = singles.tile([128, 128], FP)
        identB = singles.tile([128, 128], BF)
        epst = singles.tile([128, 1], FP)
        ones41 = singles.tile([128, 4, 1], FP)

        def bcast(name, eng):  # [D] dram -> [128, D] sbuf, partition-stride-0 DMA
            a = V1[name][:]
            t = singles.tile([128, D], FP, tag=f"bc_{name}")
            src = bass.AP(tensor=a.tensor, offset=a.offset, ap=[[0, 128]] + list(a.ap))
            eng.dma_start(out=t[:], in_=src)
            return t

        def ppart(name):  # [D] dram -> [128, NDT] sbuf (feature-on-partition)
            t = singles.tile([128, NDT], FP, tag=f"pp_{name}")
            nc.gpsimd.dma_start(out=t[:], in_=V1[name][:].rearrange("(t p) -> p t", p=128))
            return t

        # ========== phase A: loads + critical-path projections ==============
        with ExitStack() as pctx:
            mm_ps = pctx.enter_context(tc.tile_pool(name="mmps", bufs=4, space="PSUM"))
            wu_ps = pctx.enter_context(tc.tile_pool(name="wups", bufs=1, space="PSUM"))

            # Critical-first DMA order on the two HWDGE queues.
            nc.scalar.dma_start(
                out=WT[:, 0:3, :, :],
                in_=Wd[:, 0:3 * NDT * D].rearrange("p (w s d) -> p w s d", w=3, s=NDT))
            nc.sync.dma_start(
                out=KT[:, 0, :, :],
                in_=KTd[:, 0:1024].rearrange("p (s q) -> p s q", s=NDT))
            nc.scalar.dma_start(
                out=QT[:, 0, :, :],
                in_=QTd[:, 0:1024].rearrange("p (s q) -> p s q", s=NDT))
            bq_p = ppart("bq")
            bk_p = ppart("bk")
            nc.sync.dma_start(
                out=KT[:, 1, :, :],
                in_=KTd[:, 1024:2048].rearrange("p (s q) -> p s q", s=NDT))
            nc.scalar.dma_start(
                out=QT[:, 1, :, :],
                in_=QTd[:, 1024:2048].rearrange("p (s q) -> p s q", s=NDT))
            bv_b = bcast("bv", nc.gpsimd)
            bv_v = bv_b[:, :].rearrange("p (h d) -> p h d", h=H)
            nc.sync.dma_start(
                out=KT[:, 2:4, :, :],
                in_=KTd[:, 2048:4096].rearrange("p (c s q) -> p c s q", c=2, s=NDT))
            nc.scalar.dma_start(
                out=WT[:, 3, :, :],
                in_=Wd[:, 3 * NDT * D:].rearrange("p (s d) -> p s d", s=NDT))
            aq = V1["bq"][:]
            bq_b = singles.tile([128, D], FP, tag="bc_bq")
            nc.gpsimd.dma_start(
                out=bq_b[:],
                in_=bass.AP(tensor=aq.tensor, offset=aq.offset, ap=[[0, 128]] + list(aq.ap)))
            bo_b = bcast("bo", nc.gpsimd)
            if not skip_gb:
                g0_b = bcast("g0", nc.gpsimd)
                b0_b = bcast("beta0", nc.gpsimd)
                g1_b = bcast("g1", nc.gpsimd)
                b1_b = bcast("beta1", nc.gpsimd)

            # PE warm-up: dummy matmuls on garbage SBUF into a dead PSUM bank.
            # The HAM clock-gate lifts after ~3.4us of sustained PE activity;
            # these run during the DMA wait so the real projections start at
            # 2.4GHz.  No data deps -> they only occupy the in-order PE queue.
            nc.vector.memset(warm[:], 1.0)
            wu = wu_ps.tile([128, 512], FP, tag="wu")
            for _ in range(8):
                nc.tensor.matmul(wu[:], warm[:, 0:128], warm[:], start=True, stop=True)

            # constants (emitted after the DMA issues so they don't delay them)
            nc.vector.memset(ident[:], 0.0)
            make_identity(nc, ident, nomemset=True)
            nc.vector.memset(identB[:], 0.0)
            make_identity(nc, identB, nomemset=True)
            nc.vector.memset(epst, LN_EPS)
            nc.vector.memset(ones41[:], 1.0)
            nc.vector.memset(negones[:], -1.0)

            def proj_chunk(pool, dst, w, src_q, bias_p, dvt, n, on_act):
                # dst[:, dvt, n*512:(n+1)*512] = W[dvt-block] @ src + bias
                ps = pool.tile([128, 512], FP, tag=("mm" if pool is mm_ps else "fil"))
                for dqt in range(NDT):
                    nc.tensor.matmul(
                        ps[:],
                        WT[:, w, dqt, dvt * 128:(dvt + 1) * 128],
                        src_q[:, n, dqt, :],
                        start=(dqt == 0), stop=(dqt == NDT - 1))
                if on_act:
                    nc.scalar.activation(
                        out=dst[:, dvt, n * 512:(n + 1) * 512], in_=ps[:],
                        func=AF.Identity, bias=bias_p[:, dvt:dvt + 1], scale=1.0)
                else:
                    nc.vector.tensor_scalar_add(
                        out=dst[:, dvt, n * 512:(n + 1) * 512], in0=ps[:],
                        scalar1=bias_p[:, dvt:dvt + 1])

            def kproj(pool, dvt, c, on_act):
                # KpT[:, dvt, c*512:(c+1)*512]
                ps = pool.tile([128, 512], FP, tag=("mm" if pool is mm_ps else "fil"))
                for dqt in range(NDT):
                    nc.tensor.matmul(
                        ps[:],
                        WT[:, 1, dqt, dvt * 128:(dvt + 1) * 128],
                        KT[:, c, dqt, :],
                        start=(dqt == 0), stop=(dqt == NDT - 1))
                if on_act:
                    nc.scalar.activation(
                        out=KpT[:, dvt, c * 512:(c + 1) * 512], in_=ps[:],
                        func=AF.Identity, bias=bk_p[:, dvt:dvt + 1], scale=1.0)
                else:
                    nc.vector.tensor_scalar_add(
                        out=KpT[:, dvt, c * 512:(c + 1) * 512], in0=ps[:],
                        scalar1=bk_p[:, dvt:dvt + 1])

            def vp_pair(kts, pool):  # V projection for a pair of key tiles
                for kt in kts:
                    ps = pool.tile([128, 512], FP, tag=("mm" if pool is mm_ps else "fil"))
                    for dqt in range(NDT):
                        nc.tensor.matmul(
                            ps[:, :D],
                            KT[:, kt // 4, dqt, (kt % 4) * 128:(kt % 4 + 1) * 128],
                            WT[:, 2, dqt, :],
                            start=(dqt == 0), stop=(dqt == NDT - 1))
                    nc.vector.tensor_copy(out=Vp[:, kt, :, DH:DH + 1], in_=ones41[:])
                    nc.vector.tensor_add(
                        out=Vp[:, kt, :, 0:DH],
                        in0=ps[:, :D].rearrange("p (h d) -> p h d", h=H),
                        in1=bv_v)

            def obase(qt, pool):  # residual base O = Qp token-major
                ps = pool.tile([128, 512], FP, tag=("mm" if pool is mm_ps else "fil"))
                for dqt in range(NDT):
                    nc.tensor.matmul(
                        ps[:, :D],
                        QT[:, qt // 4, dqt, (qt % 4) * 128:(qt % 4 + 1) * 128],
                        WT[:, 0, dqt, :],
                        start=(dqt == 0), stop=(dqt == NDT - 1))
                nc.vector.tensor_add(out=O[:, qt, :], in0=ps[:, :D], in1=bq_b[:])

            # critical path to the first exp: QpT(dvt0 n0), KpT(dvt0 c0),
            # QpT(dvt0 n1).  Everything else is deferred to phase-B fillers.
            proj_chunk(mm_ps, QpT, 0, QT, bq_p, 0, 0, True)
            kproj(mm_ps, 0, 0, True)
            proj_chunk(mm_ps, QpT, 0, QT, bq_p, 0, 1, True)

        # ========== phase B: attention + fillers ============================
        with ExitStack() as pctx:
            sc_ps = pctx.enter_context(tc.tile_pool(name="scps", bufs=2, space="PSUM"))
            cx_ps = pctx.enter_context(tc.tile_pool(name="cxps", bufs=1, space="PSUM"))
            aux_ps = pctx.enter_context(tc.tile_pool(name="auxps", bufs=2, space="PSUM"))

            # remaining projections, drip-fed into PE slack in dependency
            # order.  obase fillers MUST be emitted before head 0's merges
            # (the merges read+write O).  Entries later in the list may
            # depend on later DMA chunks.
            # Emission order = program order: a filler pumped at iteration i
            # is emitted before ctx(kt=i) and before mm_s(kt=i+2), so
            # vp_pair((2k,2k+1)) must be pumped at iteration <= 2k-1 and
            # kproj(0,c) at iteration <= 4c-2.
            fillers = []
            fillers.append(lambda: obase(0, aux_ps))                 # h0 kt0
            fillers.append(lambda: vp_pair((4, 5), aux_ps))          # kt1
            fillers.append(lambda: kproj(aux_ps, 0, 1, False))       # kt2
            fillers.append(lambda: vp_pair((6, 7), aux_ps))          # kt3
            fillers.append(lambda: obase(1, aux_ps))                 # kt4
            fillers.append(lambda: vp_pair((8, 9), aux_ps))          # kt5
            fillers.append(lambda: kproj(aux_ps, 0, 2, False))       # kt6
            fillers.append(lambda: vp_pair((10, 11), aux_ps))        # kt7
            fillers.append(lambda: obase(2, aux_ps))                 # kt8
            fillers.append(lambda: vp_pair((12, 13), aux_ps))        # kt9
            fillers.append(lambda: kproj(aux_ps, 0, 3, False))       # kt10
            fillers.append(lambda: vp_pair((14, 15), aux_ps))        # kt11
            for qt in range(3, NQT):
                fillers.append(lambda qt=qt: obase(qt, aux_ps))      # kt12..h1 kt0
            # dvt1 projections (needed from h2) + wsum prep, during h1
            for c in range(KC):
                fillers.append(lambda c=c: kproj(aux_ps, 1, c, False))
            for n in range(QN):
                fillers.append(lambda n=n: proj_chunk(
                    aux_ps, QpT, 0, QT, bq_p, 1, n, False))

            def wsum_prep():
                # wb2 row0 = -colsum(Wo^T) (negones lhsT), row1 = bo
                wsp = aux_ps.tile([1, 256], FP, tag="fil")
                for dvt in range(NDT):
                    nc.tensor.matmul(
                        wsp[:], negones[:], WT[:, 3, dvt, :],
                        start=(dvt == 0), stop=(dvt == NDT - 1))
                nc.vector.tensor_copy(out=wb2[0:1, :], in_=wsp[:])
                bo_row1 = singles.tile([1, D], BF, tag="bo_row1")
                nc.vector.tensor_copy(out=bo_row1[:], in_=bo_b[0:1, :])
                nc.gpsimd.dma_start(out=wb2[1:2, :], in_=bo_row1[:])

            fillers.append(wsum_prep)

            def pump(n):
                for _ in range(n):
                    if fillers:
                        fillers.pop(0)()

            def mm_s(h, kt, ns=None, sps=None):
                po = (h % 2) * DH
                dvt = h // 2
                if sps is None:
                    sps = sc_ps.tile([128, SQ], FP, tag="sc")
                for n in (range(SQ // 512) if ns is None else ns):
                    nc.tensor.matmul(
                        sps[:, n * 512:(n + 1) * 512],
                        KpT[po:po + DH, dvt, kt * 128:(kt + 1) * 128],
                        QpT[po:po + DH, dvt, n * 512:(n + 1) * 512],
                        start=True, stop=True)
                return sps

            def merge_qt(h, ctxTh, qt):
                # fold head h's ctx into O for one query tile + LN0 partials
                pmt = aux_ps.tile([128, DH + 1], FP, tag="fil")
                nc.tensor.transpose(
                    pmt[:], ctxTh[:, qt * 128:(qt + 1) * 128],
                    ident[:DH + 1, :DH + 1])
                nc.vector.reciprocal_approx_fast(
                    out=recips[:, qt, h:h + 1], in_=pmt[:, DH:DH + 1])
                nc.vector.scalar_tensor_tensor(
                    out=O[:, qt, h * DH:(h + 1) * DH],
                    in0=pmt[:, 0:DH],
                    scalar=recips[:, qt, h:h + 1],
                    in1=O[:, qt, h * DH:(h + 1) * DH],
                    op0=OP.mult, op1=OP.add)
                nc.vector.bn_stats(
                    st4[:, qt, h, :], O[:, qt, h * DH:(h + 1) * DH])

            pre = None
            ctxTh_prev = None       # (h, ctxTh) whose merges are still queued
            for h in range(H):
                cps = cx_ps.tile([DH + 1, SQ], FP, tag="cx")
                if pre is None:
                    # first exp split in two halves: the n0 half starts
                    # ~1us before QpT n1's scores are even computed
                    sps, nxt_pre = mm_s(h, 0, ns=(0,)), None
                    e0 = ex.tile([128, SQ], BF, tag="ex")
                    nc.scalar.activation(
                        out=e0[:, 0:512], in_=sps[:, 0:512], func=AF.Exp, scale=SCALE)
                    mm_s(h, 0, ns=(1,), sps=sps)
                    # V projections for the first key tiles must be emitted
                    # before ctx(kt0) reads Vp (in-order emission)
                    vp_pair((0, 1), aux_ps)
                    vp_pair((2, 3), aux_ps)
                else:
                    sps, nxt_pre = pre
                    e0 = None
                for kt in range(NKT):
                    if kt == 0 and nxt_pre is not None:
                        nxt = nxt_pre
                    else:
                        nxt = mm_s(h, kt + 1) if kt + 1 < NKT else None
                    if e0 is not None:
                        e = e0
                        nc.scalar.activation(
                            out=e[:, 512:SQ], in_=sps[:, 512:SQ],
                            func=AF.Exp, scale=SCALE)
                        e0 = None
                    else:
                        e = ex.tile([128, SQ], BF, tag="ex")
                        nc.scalar.activation(
                            out=e[:], in_=sps[:], func=AF.Exp, scale=SCALE)
                    # fillers go BEFORE the ctx matmuls in the in-order PE
                    # queue: they run in the bubble while ctx waits on this
                    # exp, instead of delaying scores(kt+2).
                    pump(1)
                    for n in range(SQ // 512):
                        nc.tensor.matmul(
                            cps[:, n * 512:(n + 1) * 512],
                            Vp[:, kt, h, :],
                            e[:, n * 512:(n + 1) * 512],
                            start=(kt == 0), stop=(kt == NKT - 1))
                    sps = nxt
                    # one queued merge of the previous head per iteration
                    if ctxTh_prev is not None and 4 <= kt < 4 + NQT:
                        merge_qt(ctxTh_prev[0], ctxTh_prev[1], kt - 4)

                # pre-emit the next head's first two score-tile matmuls so
                # they run during the merge/copy window (in-order PE queue)
                if h + 1 < H:
                    pre = (mm_s(h + 1, 0), mm_s(h + 1, 1))

                # stage the ctx for merging; merges run as fillers during
                # the next head's iterations (or inline for the last head)
                ctxTh = ctp.tile([DH + 1, SQ], FP, tag="ct")
                if h == H - 1:
                    # ACT is done with exps here; 4-way split so the first
                    # query tiles' merges can start ~300ns after the last ctx
                    for i, eng in enumerate((nc.scalar, nc.vector, nc.scalar,
                                             nc.vector)):
                        if eng is nc.scalar:
                            nc.scalar.copy(
                                out=ctxTh[:, i * 256:(i + 1) * 256],
                                in_=cps[:, i * 256:(i + 1) * 256])
                        else:
                            nc.vector.tensor_copy(
                                out=ctxTh[:, i * 256:(i + 1) * 256],
                                in_=cps[:, i * 256:(i + 1) * 256])
                else:
                    nc.vector.tensor_copy(out=ctxTh[:], in_=cps[:])
                ctxTh_prev = (h, ctxTh)

        # ========== phase C: h3 merges + LN0, MLP, LN1, store ===============
        with ExitStack() as pctx:
            tr_ps = pctx.enter_context(tc.tile_pool(name="trps", bufs=2, space="PSUM"))
            wo_ps = pctx.enter_context(tc.tile_pool(name="wops", bufs=2, space="PSUM"))
            bg_ps = pctx.enter_context(tc.tile_pool(name="bgps", bufs=2, space="PSUM"))
            mg_ps = pctx.enter_context(tc.tile_pool(name="mgps", bufs=2, space="PSUM"))

            h3, ctxTh3 = ctxTh_prev

            def merge_qt_tail(qt):
                pmt = mg_ps.tile([128, DH + 1], FP, tag="mg")
                nc.tensor.transpose(
                    pmt[:], ctxTh3[:, qt * 128:(qt + 1) * 128],
                    ident[:DH + 1, :DH + 1])
                nc.vector.reciprocal_approx_fast(
                    out=recips[:, qt, h3:h3 + 1], in_=pmt[:, DH:DH + 1])
                nc.vector.scalar_tensor_tensor(
                    out=O[:, qt, h3 * DH:(h3 + 1) * DH],
                    in0=pmt[:, 0:DH],
                    scalar=recips[:, qt, h3:h3 + 1],
                    in1=O[:, qt, h3 * DH:(h3 + 1) * DH],
                    op0=OP.mult, op1=OP.add)
                nc.vector.bn_stats(
                    st4[:, qt, h3, :], O[:, qt, h3 * DH:(h3 + 1) * DH])
                nc.vector.bn_aggr(msd[:, qt, :], st4[:, qt, :, :])

            def sd_group(qb, k):
                # msd[:, qb:qb+k, 1]: v0 -> sd0 = exp(0.5*ln(v0+eps))
                nc.scalar.activation(
                    out=lnt[:, qb:qb + k], in_=msd[:, qb:qb + k, 1],
                    func=AF.Ln, bias=epst[:], scale=1.0)
                nc.scalar.activation(
                    out=msd[:, qb:qb + k, 1], in_=lnt[:, qb:qb + k],
                    func=AF.Exp, scale=0.5)

            # interleaved per-qt pipeline; merges first (2-qt lookahead),
            # sd conversion per 2-qt group, then the MLP/LN chain.
            for qt in range(NQT):
                if qt == 0:
                    merge_qt_tail(0)
                    merge_qt_tail(1)
                    sd_group(0, 2)
                # O transposes (bf16, 1-pass) -> OT
                tr = tr_ps.tile([128, 256], BF, tag="tr")
                for dvt in range(NDT):
                    nc.tensor.transpose(
                        tr[:, dvt * 128:(dvt + 1) * 128],
                        O[:, qt, dvt * 128:(dvt + 1) * 128], identB[:])
                nc.scalar.copy(
                    out=OT[:, :, qt * 128:(qt + 1) * 128],
                    in_=tr[:, :].rearrange("p (t q) -> p t q", t=NDT))
                # rank-2 correction rows [m; sd] -> bgrow
                bgp = bg_ps.tile([2, 128], FP, tag="bg")
                nc.tensor.transpose(bgp[:], msd[:, qt, :], ident[:])
                nc.scalar.copy(out=bgrow[:, qt, :], in_=bgp[:])
                # Wo matmul + correction
                wo = wo_ps.tile([128, 256], FP, tag="wo")
                for dvt in range(NDT):
                    nc.tensor.matmul(
                        wo[:], OT[:, dvt, qt * 128:(qt + 1) * 128],
                        WT[:, 3, dvt, :], start=(dvt == 0), stop=False)
                nc.tensor.matmul(
                    wo[:], bgrow[:, qt, :], wb2[:], start=False, stop=True)
                # z = O + relu(p4)   (fused on vector)
                zt = ztp.tile([128, D], FP, tag="zt")
                nc.vector.scalar_tensor_tensor(
                    out=zt[:], in0=wo[:], scalar=0.0, in1=O[:, qt, :],
                    op0=OP.max, op1=OP.add)
                st = ztp.tile([128, 6], FP, tag="st")
                nc.vector.bn_stats(st[:], zt[:])
                nc.vector.bn_aggr(mv1[:, qt, :], st[:])
                # pipeline: next pair of merges + sd while this qt's LN1
                # stats settle
                if qt + 2 < NQT and qt % 2 == 0:
                    merge_qt_tail(qt + 2)
                    merge_qt_tail(qt + 3)
                    sd_group(qt + 2, 2)
                # rs1 = exp(-0.5*ln(v1+eps)); normalize; store
                nc.scalar.activation(
                    out=lnt[:, qt:qt + 1], in_=mv1[:, qt:qt + 1, 1],
                    func=AF.Ln, bias=epst[:], scale=1.0)
                nc.scalar.activation(
                    out=rs1[:, qt:qt + 1], in_=lnt[:, qt:qt + 1],
                    func=AF.Exp, scale=-0.5)
                f = outp.tile([128, D], BF, tag="f")
                nc.vector.tensor_scalar(
                    out=f[:], in0=zt[:],
                    scalar1=mv1[:, qt, 0:1], scalar2=rs1[:, qt:qt + 1],
                    op0=OP.subtract, op1=OP.mult)
                if not skip_gb:
                    nc.vector.tensor_mul(out=f[:], in0=f[:], in1=g1_b[:])
                    nc.vector.tensor_add(out=f[:], in0=f[:], in1=b1_b[:])
                deng = (nc.sync, nc.scalar)[qt % 2]
                deng.dma_start(out=out[qt * 128:(qt + 1) * 128, :], in_=f[:])

    return nc


_NC = {}


def build_nc(skip_gb=True):
    if skip_gb not in _NC:
        _patch_act_tables()
        nc = bacc.Bacc("TRN2", target_bir_lowering=False)
        _emit(nc, skip_gb)
        nc.compile()
        _NC[skip_gb] = nc
    return _NC[skip_gb]


def shard_inputs(Q, K, Wq, bq, Wk, bk, Wv, bv, Wo, bo, g0, beta0, g1, beta1):
    # host-side zero-FLOP layout transforms: ship everything feature-major bf16
    bf = ml_dtypes.bfloat16

    def wshape(w):  # [D, D] -> partition-major [128, NDT*D] (contiguous rows)
        wt = np.asarray(w).T.astype(bf)           # [ (s p), d ]
        return np.ascontiguousarray(
            wt.reshape(NDT, 128, D).transpose(1, 0, 2).reshape(128, NDT * D))

    def xshape(x, nblk):  # [S, D] -> [128, nblk, NDT, 512] -> [128, nblk*NDT*512]
        xt = np.asarray(x).T.astype(bf)           # [(s p), (n q)]
        return np.ascontiguousarray(
            xt.reshape(NDT, 128, nblk, 512).transpose(1, 2, 0, 3).reshape(128, -1))

    shared = {
        "Wall": np.ascontiguousarray(np.concatenate(
            [wshape(Wq), wshape(Wk), wshape(Wv), wshape(Wo)], axis=1)),
    }
    for n, v in (("bq", bq), ("bk", bk), ("bv", bv), ("bo", bo),
                 ("g0", g0), ("beta0", beta0), ("g1", g1), ("beta1", beta1)):
        shared[n] = np.ascontiguousarray(np.asarray(v, dtype=np.float32))
    in_maps = []
    for c in range(NCORES):
        b, half = c // QSPLIT, c % QSPLIT
        m = dict(shared)
        m["QT"] = xshape(Q[b, half * SQ:(half + 1) * SQ, :], QN)
        m["KT"] = xshape(K[b], KC)
        in_maps.append(m)
    return in_maps


def _gb_trivial(g0, beta0, g1, beta1):
    return bool(
        np.all(np.asarray(g0) == 1) and np.all(np.asarray(beta0) == 0)
        and np.all(np.asarray(g1) == 1) and np.all(np.asarray(beta1) == 0))


def kernel(**inputs):
    skip_gb = _gb_trivial(inputs["g0"], inputs["beta0"], inputs["g1"], inputs["beta1"])
    nc = build_nc(skip_gb)
    in_maps = shard_inputs(**inputs)
    res = run_bass_kernel_spmd(nc, in_maps, core_ids=list(range(NCORES)))
    out = np.empty((B, SQ_FULL, D), np.float32)
    for c in range(NCORES):
        b, half = c // QSPLIT, c % QSPLIT
        out[b, half * SQ:(half + 1) * SQ, :] = res.results[c]["out"]
    return out


# revision 27
# speedup vs baseline: 1.2096x; 1.2096x over previous
"""Trainium2 Bass kernel for nn_AttentionBlock (Set-Transformer MAB block).

Reference computation (per batch b):
    Qp = Q @ Wq.T + bq ; Kp = K @ Wk.T + bk ; Vp = K @ Wv.T + bv   (4 heads of 64)
    A  = softmax(Qp Kp^T / 8)  ;  ctx = A Vp
    O  = LN0(Qp + ctx) ;  O = O + relu(O @ Wo.T + bo) ;  out = LN1(O)

Sharding: data-parallel over (batch, query-half) -> 8 independent shards,
one per NeuronCore, no collectives.  Each core sees its 1024 queries, the
full 2048 keys of its batch, and all weights, shipped feature-major bf16.

v2 design notes (changes vs the first working kernel):
  * ACT exp is the pacing resource: 64 x [128,1024] exps ~ 71us.  Target
    is exp-stream density: startup to first exp minimized, no ACT table
    switches (we monkeypatch the act-table list so the single
    natural_log_exp_and_others set serves Exp/Ln/Identity/Relu/Copy),
    and the tail overlaps the last heads' merges.
  * DMA: inputs shipped as [128, N] partition-major tensors with 2-8KB
    contiguous rows; critical tensors (Wqkv, QT, KT chunk0) issued first
    on the two HWDGE queues.  PE warm-up dummies run during the DMA wait
    so the HAM clock-gate lifts (1.2 -> 2.4GHz) before the projections.
  * tail math: LayerNorm is invariant to per-token affine maps and relu
    is positively homogeneous, so with m/sd from bn_stats:
        z   = O + sd*relu(rs*(O@Wo^T - m*wsum + sd*bo)) = O + relu(p4)
        out = (z - m1) / sd1
    where p4 rides the Wo matmul as one K=2 correction matmul with
    stationary rows [m; sd] against [-wsum; bo].  sqrt is computed as
    exp(0.5*ln(v+eps)) on ACT (same table set as the softmax exps), and
    1/sd1 as exp(-0.5*ln(v1+eps)) -- no Sqrt table switch, no vector
    reciprocal on the critical path.
  * O is kept bf16 (transposes are 1-pass on the PE); the z residual is
    fp32.  Tail work is split ACT/Vector/PE to pipeline ~1.4us/query-tile.
"""

from contextlib import ExitStack

import ml_dtypes
import numpy as np

import concourse.bass as bass
import concourse.tile as tile
from concourse import bacc, mybir
from concourse.bass_utils import run_bass_kernel_spmd
from concourse.masks import make_identity

FP = mybir.dt.float32
BF = mybir.dt.bfloat16
AF = mybir.ActivationFunctionType
OP = mybir.AluOpType

B = 4
SQ_FULL = 2048   # queries per batch
SK = 2048        # keys per batch
D = 256
H = 4
DH = D // H      # 64
NCORES = 8
QSPLIT = 2
SQ = SQ_FULL // QSPLIT    # queries per core
NQT = SQ // 128           # 8 query tiles
NKT = SK // 128           # 16 key tiles
NDT = D // 128            # 2 feature tiles
QN = SQ // 512            # 2 query column blocks
KC = SK // 512            # 4 key column chunks
LN_EPS = 1e-5
SCALE = 0.125             # 1 / sqrt(DH)

def _emit(nc):
    # DRAM parameters: everything partition-major with fat contiguous rows.
    QTd = nc.declare_dram_parameter("QT", [128, QN * NDT * 512], BF, isOutput=False)
    KTd = nc.declare_dram_parameter("KT", [128, KC * NDT * 512], BF, isOutput=False)
    Wd = nc.declare_dram_parameter("Wall", [128, 4 * NDT * D], BF, isOutput=False)
    Browd = nc.declare_dram_parameter("brows", [1, 3 * D], BF, isOutput=False)
    V1 = {
        n: nc.declare_dram_parameter(n, [D], FP, isOutput=False)
        for n in ("bq", "bk")
    }
    out = nc.declare_dram_parameter("out", [SQ, D], BF, isOutput=True)

    with tile.TileContext(nc) as tc, ExitStack() as ctx:
        singles = ctx.enter_context(tc.tile_pool(name="singles", bufs=1))
        big = ctx.enter_context(tc.tile_pool(name="big", bufs=1))
        ex = ctx.enter_context(tc.tile_pool(name="ex", bufs=4))
        ctp = ctx.enter_context(tc.tile_pool(name="ctp", bufs=2))
        outp = ctx.enter_context(tc.tile_pool(name="outp", bufs=8))
        ztp = ctx.enter_context(tc.tile_pool(name="ztp", bufs=4))

        QpT = big.tile([128, NDT, SQ], BF)
        KpT = big.tile([128, NDT, SK], BF)
        Vp = big.tile([128, NKT, H, DH + 1], BF)
        O = big.tile([128, NQT, D], BF)
        OT = big.tile([128, NDT, SQ], BF)
        recips = big.tile([128, NQT, H], FP)
        KT = big.tile([128, KC, NDT, 512], BF)
        QT = big.tile([128, QN, NDT, 512], BF)
        WT = big.tile([128, 4, NDT, D], BF)     # Wq | Wk | Wv | Wo
        # tail stats
        msd = big.tile([128, NQT, 2], FP, tag="msd")    # [m0, v0->sd0]
        mv1 = big.tile([128, NQT, 2], FP, tag="mv1")
        rs1 = big.tile([128, NQT], FP, tag="rs1")
        lnt = big.tile([128, NQT], FP, tag="lnt")
        st4 = big.tile([128, NQT, H, 6], FP, tag="st4")
        bgrow = big.tile([2, NQT, 128], BF, tag="bgrow")
        wb2 = singles.tile([2, D], BF, tag="wb2")
        negones = singles.tile([128, 1], BF, tag="negones")
        ones_row = singles.tile([1, 128], BF, tag="ones_row")
        warm = singles.tile([128, 512], BF, tag="warm")  # PE warm-up src

        ident = singles.tile([128, 128], FP)
        identB = singles.tile([128, 128], BF)
        epst = singles.tile([128, 1], FP)
        ones41 = singles.tile([128, 4, 1], FP)

        def ppart(name):  # [D] dram -> [128, NDT] sbuf (feature-on-partition)
            t = singles.tile([128, NDT], FP, tag=f"pp_{name}")
            nc.gpsimd.dma_start(out=t[:], in_=V1[name][:].rearrange("(t p) -> p t", p=128))
            return t

        # ========== phase A: loads + critical-path projections ==============
        with ExitStack() as pctx:
            mm_ps = pctx.enter_context(tc.tile_pool(name="mmps", bufs=4, space="PSUM"))
            wu_ps = pctx.enter_context(tc.tile_pool(name="wups", bufs=1, space="PSUM"))

            # PE warm-up first: dummy matmuls on a memset tile into a dead
            # PSUM bank.  The HAM clock-gate lifts after ~3.4us of sustained
            # PE activity; the gpsimd memset lands ~6.5us (before the DMA
            # issues), so the PE is at 2.4GHz by ~10us when the projections
            # start.  No data deps -> they only occupy the in-order PE queue.
            nc.gpsimd.memset(warm[:], 1.0)
            wu = wu_ps.tile([128, 512], FP, tag="wu")
            for _ in range(8):
                nc.tensor.matmul(wu[:], warm[:, 0:128], warm[:], start=True, stop=True)

            # Critical-first DMA order, contention-controlled: the 16 SDMA
            # engines round-robin among all in-flight transfers, so only the
            # critical ones (Wqkv, KT chunk0, QT) are issued up front; the
            # tiny bias-row load sits between KTc0 and KTc1 on the sync
            # queue, delaying the non-critical KT chunks ~1us each.
            nc.scalar.dma_start(
                out=WT[:, 0:3, :, :],
                in_=Wd[:, 0:3 * NDT * D].rearrange("p (w s d) -> p w s d", w=3, s=NDT))
            nc.sync.dma_start(
                out=KT[:, 0, :, :],
                in_=KTd[:, 0:1024].rearrange("p (s q) -> p s q", s=NDT))
            nc.scalar.dma_start(
                out=QT[:, 0, :, :],
                in_=QTd[:, 0:1024].rearrange("p (s q) -> p s q", s=NDT))
            brow = singles.tile([1, 3 * D], BF, tag="brow")  # bq | bv | bo rows
            nc.sync.dma_start(out=brow[:], in_=Browd[:, :])
            bq_p = ppart("bq")
            bk_p = ppart("bk")
            nc.scalar.dma_start(
                out=QT[:, 1, :, :],
                in_=QTd[:, 1024:2048].rearrange("p (s q) -> p s q", s=NDT))
            nc.sync.dma_start(
                out=KT[:, 1, :, :],
                in_=KTd[:, 1024:2048].rearrange("p (s q) -> p s q", s=NDT))
            nc.scalar.dma_start(
                out=WT[:, 3, :, :],
                in_=Wd[:, 3 * NDT * D:].rearrange("p (s d) -> p s d", s=NDT))
            nc.sync.dma_start(
                out=KT[:, 2:4, :, :],
                in_=KTd[:, 2048:4096].rearrange("p (c s q) -> p c s q", c=2, s=NDT))

            # constants (emitted after the DMA issues so they don't delay them)
            nc.vector.memset(ident[:], 0.0)
            make_identity(nc, ident, nomemset=True)
            nc.vector.memset(identB[:], 0.0)
            make_identity(nc, identB, nomemset=True)
            nc.vector.memset(epst, LN_EPS)
            nc.vector.memset(ones41[:], 1.0)
            nc.vector.memset(negones[:], -1.0)
            nc.vector.memset(ones_row[:], 1.0)

            # token-major bias broadcasts via rank-1 matmuls (replaces
            # 384KB of stride-0 HBM broadcast reads with a 1.5KB row load)
            bq_b = singles.tile([128, D], FP, tag="bb_bq")
            bv_b = singles.tile([128, D], FP, tag="bb_bv")
            for row, dst in ((0, bq_b), (1, bv_b)):
                bps = mm_ps.tile([128, 512], FP, tag="mm")
                nc.tensor.matmul(
                    bps[:, :D], ones_row[:], brow[0:1, row * D:(row + 1) * D],
                    start=True, stop=True)
                nc.vector.tensor_copy(out=dst[:], in_=bps[:, :D])
            bv_v = bv_b[:, :].rearrange("p (h d) -> p h d", h=H)

            def proj_chunk(pool, dst, w, src_q, bias_p, dvt, n, on_act):
                # dst[:, dvt, n*512:(n+1)*512] = W[dvt-block] @ src + bias
                ps = pool.tile([128, 512], FP, tag=("mm" if pool is mm_ps else "fil"))
                for dqt in range(NDT):
                    nc.tensor.matmul(
                        ps[:],
                        WT[:, w, dqt, dvt * 128:(dvt + 1) * 128],
                        src_q[:, n, dqt, :],
                        start=(dqt == 0), stop=(dqt == NDT - 1))
                if on_act:
                    nc.scalar.activation(
                        out=dst[:, dvt, n * 512:(n + 1) * 512], in_=ps[:],
                        func=AF.Identity, bias=bias_p[:, dvt:dvt + 1], scale=1.0)
                else:
                    nc.vector.tensor_scalar_add(
                        out=dst[:, dvt, n * 512:(n + 1) * 512], in0=ps[:],
                        scalar1=bias_p[:, dvt:dvt + 1])

            def kproj(pool, dvt, c, on_act):
                # KpT[:, dvt, c*512:(c+1)*512]
                ps = pool.tile([128, 512], FP, tag=("mm" if pool is mm_ps else "fil"))
                for dqt in range(NDT):
                    nc.tensor.matmul(
                        ps[:],
                        WT[:, 1, dqt, dvt * 128:(dvt + 1) * 128],
                        KT[:, c, dqt, :],
                        start=(dqt == 0), stop=(dqt == NDT - 1))
                if on_act:
                    nc.scalar.activation(
                        out=KpT[:, dvt, c * 512:(c + 1) * 512], in_=ps[:],
                        func=AF.Identity, bias=bk_p[:, dvt:dvt + 1], scale=1.0)
                else:
                    nc.vector.tensor_scalar_add(
                        out=KpT[:, dvt, c * 512:(c + 1) * 512], in0=ps[:],
                        scalar1=bk_p[:, dvt:dvt + 1])

            def vp_pair(kts, pool):  # V projection for a pair of key tiles
                for kt in kts:
                    ps = pool.tile([128, 512], FP, tag=("mm" if pool is mm_ps else "fil"))
                    for dqt in range(NDT):
                        nc.tensor.matmul(
                            ps[:, :D],
                            KT[:, kt // 4, dqt, (kt % 4) * 128:(kt % 4 + 1) * 128],
                            WT[:, 2, dqt, :],
                            start=(dqt == 0), stop=(dqt == NDT - 1))
                    nc.vector.tensor_copy(out=Vp[:, kt, :, DH:DH + 1], in_=ones41[:])
                    nc.vector.tensor_add(
                        out=Vp[:, kt, :, 0:DH],
                        in0=ps[:, :D].rearrange("p (h d) -> p h d", h=H),
                        in1=bv_v)

            def obase(qt, pool):  # residual base O = Qp token-major
                ps = pool.tile([128, 512], FP, tag=("mm" if pool is mm_ps else "fil"))
                for dqt in range(NDT):
                    nc.tensor.matmul(
                        ps[:, :D],
                        QT[:, qt // 4, dqt, (qt % 4) * 128:(qt % 4 + 1) * 128],
                        WT[:, 0, dqt, :],
                        start=(dqt == 0), stop=(dqt == NDT - 1))
                nc.vector.tensor_add(out=O[:, qt, :], in0=ps[:, :D], in1=bq_b[:])

            # critical path to the first exp: QpT(dvt0 n0), KpT(dvt0 c0),
            # QpT(dvt0 n1).  The Kp bias add goes to Vector so it runs in
            # parallel with the Qp identity on ACT.
            proj_chunk(mm_ps, QpT, 0, QT, bq_p, 0, 0, True)
            kproj(mm_ps, 0, 0, False)
            proj_chunk(mm_ps, QpT, 0, QT, bq_p, 0, 1, True)

        # ========== phase B: attention + fillers ============================
        with ExitStack() as pctx:
            sc_ps = pctx.enter_context(tc.tile_pool(name="scps", bufs=2, space="PSUM"))
            cx_ps = pctx.enter_context(tc.tile_pool(name="cxps", bufs=1, space="PSUM"))
            aux_ps = pctx.enter_context(tc.tile_pool(name="auxps", bufs=2, space="PSUM"))

            # remaining projections, drip-fed into PE slack in dependency
            # order.  obase fillers MUST be emitted before head 0's merges
            # (the merges read+write O).  Entries later in the list may
            # depend on later DMA chunks.
            # Emission order = program order: a filler pumped at iteration i
            # is emitted before ctx(kt=i) and before mm_s(kt=i+2), so
            # vp_pair((2k,2k+1)) must be pumped at iteration <= 2k-1 and
            # kproj(0,c) at iteration <= 4c-2.
            fillers = []
            fillers.append(lambda: obase(0, aux_ps))                 # h0 kt0
            fillers.append(lambda: vp_pair((4, 5), aux_ps))          # kt1
            fillers.append(lambda: kproj(aux_ps, 0, 1, False))       # kt2
            fillers.append(lambda: vp_pair((6, 7), aux_ps))          # kt3
            fillers.append(lambda: obase(1, aux_ps))                 # kt4
            fillers.append(lambda: vp_pair((8, 9), aux_ps))          # kt5
            fillers.append(lambda: kproj(aux_ps, 0, 2, False))       # kt6
            fillers.append(lambda: vp_pair((10, 11), aux_ps))        # kt7
            fillers.append(lambda: obase(2, aux_ps))                 # kt8
            fillers.append(lambda: vp_pair((12, 13), aux_ps))        # kt9
            fillers.append(lambda: kproj(aux_ps, 0, 3, False))       # kt10
            fillers.append(lambda: vp_pair((14, 15), aux_ps))        # kt11
            for qt in range(3, NQT):
                fillers.append(lambda qt=qt: obase(qt, aux_ps))      # kt12..h1 kt0
            # dvt1 projections (needed from h2) + wsum prep, during h1
            for c in range(KC):
                fillers.append(lambda c=c: kproj(aux_ps, 1, c, False))
            for n in range(QN):
                fillers.append(lambda n=n: proj_chunk(
                    aux_ps, QpT, 0, QT, bq_p, 1, n, False))

            def wsum_prep():
                # wb2 row0 = -colsum(Wo^T) (negones lhsT), row1 = bo.
                # engines can't address a base partition of 1 -> wb2 row1
                # goes through a tiny SBUF->SBUF DMA.
                wsp = aux_ps.tile([1, 256], FP, tag="fil")
                for dvt in range(NDT):
                    nc.tensor.matmul(
                        wsp[:], negones[:], WT[:, 3, dvt, :],
                        start=(dvt == 0), stop=(dvt == NDT - 1))
                nc.vector.tensor_copy(out=wb2[0:1, :], in_=wsp[:])
                nc.gpsimd.dma_start(out=wb2[1:2, :], in_=brow[0:1, 2 * D:3 * D])

            fillers.append(wsum_prep)

            def pump(n):
                for _ in range(n):
                    if fillers:
                        fillers.pop(0)()

            def mm_s(h, kt, ns=None, sps=None):
                po = (h % 2) * DH
                dvt = h // 2
                if sps is None:
                    sps = sc_ps.tile([128, SQ], FP, tag="sc")
                for n in (range(SQ // 512) if ns is None else ns):
                    nc.tensor.matmul(
                        sps[:, n * 512:(n + 1) * 512],
                        KpT[po:po + DH, dvt, kt * 128:(kt + 1) * 128],
                        QpT[po:po + DH, dvt, n * 512:(n + 1) * 512],
                        start=True, stop=True)
                return sps

            def merge_qt(h, ctxTh, qt):
                # fold head h's ctx into O for one query tile + LN0 partials
                pmt = aux_ps.tile([128, DH + 1], FP, tag="fil")
                nc.tensor.transpose(
                    pmt[:], ctxTh[:, qt * 128:(qt + 1) * 128],
                    ident[:DH + 1, :DH + 1])
                nc.vector.reciprocal_approx_fast(
                    out=recips[:, qt, h:h + 1], in_=pmt[:, DH:DH + 1])
                nc.vector.scalar_tensor_tensor(
                    out=O[:, qt, h * DH:(h + 1) * DH],
                    in0=pmt[:, 0:DH],
                    scalar=recips[:, qt, h:h + 1],
                    in1=O[:, qt, h * DH:(h + 1) * DH],
                    op0=OP.mult, op1=OP.add)
                nc.vector.bn_stats(
                    st4[:, qt, h, :], O[:, qt, h * DH:(h + 1) * DH])

            pre = None
            ctxTh_prev = None       # (h, ctxTh) whose merges are still queued
            for h in range(H):
                cps = cx_ps.tile([DH + 1, SQ], FP, tag="cx")
                if pre is None:
                    # first exp split in two halves: the n0 half starts
                    # ~1us before QpT n1's scores are even computed
                    sps, nxt_pre = mm_s(h, 0, ns=(0,)), None
                    e0 = ex.tile([128, SQ], BF, tag="ex")
                    nc.scalar.activation(
                        out=e0[:, 0:512], in_=sps[:, 0:512], func=AF.Exp, scale=SCALE)
                    mm_s(h, 0, ns=(1,), sps=sps)
                    # V projections for the first key tiles must be emitted
                    # before ctx(kt0) reads Vp (in-order emission)
                    vp_pair((0, 1), aux_ps)
                    vp_pair((2, 3), aux_ps)
                else:
                    sps, nxt_pre = pre
                    e0 = None
                for kt in range(NKT):
                    if kt == 0 and nxt_pre is not None:
                        nxt = nxt_pre
                    else:
                        nxt = mm_s(h, kt + 1) if kt + 1 < NKT else None
                    if e0 is not None:
                        e = e0
                        nc.scalar.activation(
                            out=e[:, 512:SQ], in_=sps[:, 512:SQ],
                            func=AF.Exp, scale=SCALE)
                        e0 = None
                    else:
                        e = ex.tile([128, SQ], BF, tag="ex")
                        nc.scalar.activation(
                            out=e[:], in_=sps[:], func=AF.Exp, scale=SCALE)
                    # one PE-side side-task per iteration, BEFORE the ctx
                    # matmuls in the in-order PE queue: it runs in the bubble
                    # while ctx waits on this exp, instead of delaying
                    # scores(kt+2).  Merges of the previous head take the odd
                    # iterations, projection fillers the even ones — two
                    # tasks in one iteration overloads the PE beyond the exp
                    # pace (~1.35us/kt > 1.11us).
                    if ctxTh_prev is not None and kt % 2 == 1:
                        merge_qt(ctxTh_prev[0], ctxTh_prev[1], kt // 2)
                    else:
                        pump(1)
                    for n in range(SQ // 512):
                        nc.tensor.matmul(
                            cps[:, n * 512:(n + 1) * 512],
                            Vp[:, kt, h, :],
                            e[:, n * 512:(n + 1) * 512],
                            start=(kt == 0), stop=(kt == NKT - 1))
                    sps = nxt

                # pre-emit the next head's first two score-tile matmuls so
                # they run during the merge/copy window (in-order PE queue)
                if h + 1 < H:
                    pre = (mm_s(h + 1, 0), mm_s(h + 1, 1))

                # stage the ctx for merging; merges run as fillers during
                # the next head's iterations (or inline for the last head)
                ctxTh = ctp.tile([DH + 1, SQ], FP, tag="ct")
                if h == H - 1:
                    # ACT is done with exps here; 4-way split so the first
                    # query tiles' merges can start ~300ns after the last ctx
                    for i, eng in enumerate((nc.scalar, nc.vector, nc.scalar,
                                             nc.vector)):
                        if eng is nc.scalar:
                            nc.scalar.copy(
                                out=ctxTh[:, i * 256:(i + 1) * 256],
                                in_=cps[:, i * 256:(i + 1) * 256])
                        else:
                            nc.vector.tensor_copy(
                                out=ctxTh[:, i * 256:(i + 1) * 256],
                                in_=cps[:, i * 256:(i + 1) * 256])
                else:
                    nc.vector.tensor_copy(out=ctxTh[:], in_=cps[:])
                ctxTh_prev = (h, ctxTh)

        # ========== phase C: h3 merges + LN0, MLP, LN1, store ===============
        with ExitStack() as pctx:
            tr_ps = pctx.enter_context(tc.tile_pool(name="trps", bufs=2, space="PSUM"))
            wo_ps = pctx.enter_context(tc.tile_pool(name="wops", bufs=2, space="PSUM"))
            bg_ps = pctx.enter_context(tc.tile_pool(name="bgps", bufs=2, space="PSUM"))
            mg_ps = pctx.enter_context(tc.tile_pool(name="mgps", bufs=2, space="PSUM"))

            h3, ctxTh3 = ctxTh_prev

            def merge_qt_tail(qt):
                pmt = mg_ps.tile([128, DH + 1], FP, tag="mg")
                nc.tensor.transpose(
                    pmt[:], ctxTh3[:, qt * 128:(qt + 1) * 128],
                    ident[:DH + 1, :DH + 1])
                nc.vector.reciprocal_approx_fast(
                    out=recips[:, qt, h3:h3 + 1], in_=pmt[:, DH:DH + 1])
                nc.vector.scalar_tensor_tensor(
                    out=O[:, qt, h3 * DH:(h3 + 1) * DH],
                    in0=pmt[:, 0:DH],
                    scalar=recips[:, qt, h3:h3 + 1],
                    in1=O[:, qt, h3 * DH:(h3 + 1) * DH],
                    op0=OP.mult, op1=OP.add)
                nc.vector.bn_stats(
                    st4[:, qt, h3, :], O[:, qt, h3 * DH:(h3 + 1) * DH])
                nc.vector.bn_aggr(msd[:, qt, :], st4[:, qt, :, :])

            def sd_group(qb, k):
                # msd[:, qb:qb+k, 1]: v0 -> sd0 = sqrt(v0 + eps), in place
                # (elementwise same-range in/out is stream-safe on ACT)
                nc.scalar.activation(
                    out=msd[:, qb:qb + k, 1], in_=msd[:, qb:qb + k, 1],
                    func=AF.Sqrt, bias=epst[:], scale=1.0)

            # interleaved per-qt pipeline; merges first (2-qt lookahead),
            # sd conversion per 2-qt group, then the MLP/LN chain.
            for qt in range(NQT):
                if qt == 0:
                    merge_qt_tail(0)
                    merge_qt_tail(1)
                    sd_group(0, 2)
                # O transposes (bf16, 1-pass) -> OT
                tr = tr_ps.tile([128, 256], BF, tag="tr")
                for dvt in range(NDT):
                    nc.tensor.transpose(
                        tr[:, dvt * 128:(dvt + 1) * 128],
                        O[:, qt, dvt * 128:(dvt + 1) * 128], identB[:])
                nc.scalar.copy(
                    out=OT[:, :, qt * 128:(qt + 1) * 128],
                    in_=tr[:, :].rearrange("p (t q) -> p t q", t=NDT))
                # rank-2 correction rows [m; sd] -> bgrow
                bgp = bg_ps.tile([2, 128], FP, tag="bg")
                nc.tensor.transpose(bgp[:], msd[:, qt, :], ident[:])
                nc.scalar.copy(out=bgrow[:, qt, :], in_=bgp[:])
                # Wo matmul + correction
                wo = wo_ps.tile([128, 256], FP, tag="wo")
                for dvt in range(NDT):
                    nc.tensor.matmul(
                        wo[:], OT[:, dvt, qt * 128:(qt + 1) * 128],
                        WT[:, 3, dvt, :], start=(dvt == 0), stop=False)
                nc.tensor.matmul(
                    wo[:], bgrow[:, qt, :], wb2[:], start=False, stop=True)
                # z = O + relu(p4)   (fused on vector)
                zt = ztp.tile([128, D], FP, tag="zt")
                nc.vector.scalar_tensor_tensor(
                    out=zt[:], in0=wo[:], scalar=0.0, in1=O[:, qt, :],
                    op0=OP.max, op1=OP.add)
                st = ztp.tile([128, 6], FP, tag="st")
                nc.vector.bn_stats(st[:], zt[:])
                nc.vector.bn_aggr(mv1[:, qt, :], st[:])
                # pipeline: next pair of merges + sd while this qt's LN1
                # stats settle
                if qt + 2 < NQT and qt % 2 == 0:
                    merge_qt_tail(qt + 2)
                    merge_qt_tail(qt + 3)
                    sd_group(qt + 2, 2)
                # rs1 = 1/sqrt(v1+eps); normalize; store
                nc.scalar.activation(
                    out=lnt[:, qt:qt + 1], in_=mv1[:, qt:qt + 1, 1],
                    func=AF.Sqrt, bias=epst[:], scale=1.0)
                nc.vector.reciprocal_approx_fast(
                    out=rs1[:, qt:qt + 1], in_=lnt[:, qt:qt + 1])
                f = outp.tile([128, D], BF, tag="f")
                nc.vector.tensor_scalar(
                    out=f[:], in0=zt[:],
                    scalar1=mv1[:, qt, 0:1], scalar2=rs1[:, qt:qt + 1],
                    op0=OP.subtract, op1=OP.mult)
                deng = (nc.sync, nc.scalar)[qt % 2]
                deng.dma_start(out=out[qt * 128:(qt + 1) * 128, :], in_=f[:])

    return nc


_NC = {}


def build_nc():
    # NOTE: no act-table pinning — natural_log_exp_and_others measurably
    # slows every ACTIVATE by ~200ns (~+15us on the exp stream).  Default
    # greedy sets: Exp/Identity/Copy -> exp_and_others, Sqrt ->
    # sqrt_and_others with exactly one switch after the last exp.
    if "nc" not in _NC:
        nc = bacc.Bacc("TRN2", target_bir_lowering=False)
        _emit(nc)
        nc.compile()
        _NC["nc"] = nc
    return _NC["nc"]


def shard_inputs(Q, K, Wq, bq, Wk, bk, Wv, bv, Wo, bo, g0, beta0, g1, beta1):
    # host-side zero-FLOP layout transforms: ship everything feature-major bf16
    bf = ml_dtypes.bfloat16

    def wshape(w):  # [D, D] -> partition-major [128, NDT*D] (contiguous rows)
        wt = np.asarray(w).T.astype(bf)           # [ (s p), d ]
        return np.ascontiguousarray(
            wt.reshape(NDT, 128, D).transpose(1, 0, 2).reshape(128, NDT * D))

    def xshape(x, nblk):  # [S, D] -> [128, nblk, NDT, 512] -> [128, nblk*NDT*512]
        xt = np.asarray(x).T.astype(bf)           # [(s p), (n q)]
        return np.ascontiguousarray(
            xt.reshape(NDT, 128, nblk, 512).transpose(1, 2, 0, 3).reshape(128, -1))

    shared = {
        "Wall": np.ascontiguousarray(np.concatenate(
            [wshape(Wq), wshape(Wk), wshape(Wv), wshape(Wo)], axis=1)),
        "brows": np.ascontiguousarray(np.concatenate(
            [np.asarray(v, dtype=np.float32) for v in (bq, bv, bo)]
        ).astype(bf).reshape(1, 3 * D)),
        "bq": np.ascontiguousarray(np.asarray(bq, dtype=np.float32)),
        "bk": np.ascontiguousarray(np.asarray(bk, dtype=np.float32)),
    }
    in_maps = []
    for c in range(NCORES):
        b, half = c // QSPLIT, c % QSPLIT
        m = dict(shared)
        m["QT"] = xshape(Q[b, half * SQ:(half + 1) * SQ, :], QN)
        m["KT"] = xshape(K[b], KC)
        in_maps.append(m)
    return in_maps


def _gb_trivial(g0, beta0, g1, beta1):
    return bool(
        np.all(np.asarray(g0) == 1) and np.all(np.asarray(beta0) == 0)
        and np.all(np.asarray(g1) == 1) and np.all(np.asarray(beta1) == 0))


def _kernel_numpy(Q, K, Wq, bq, Wk, bk, Wv, bv, Wo, bo, g0, beta0, g1, beta1):
    # general gamma/beta fallback (the device pipeline folds LN affines away,
    # which is only valid for g=1, beta=0 — the shapes this problem ships)
    def ln(x, g, b):
        m = x.mean(-1, keepdims=True)
        v = ((x - m) ** 2).mean(-1, keepdims=True)
        return (x - m) / np.sqrt(v + LN_EPS) * g + b

    Qf = np.asarray(Q, np.float32)
    Kf = np.asarray(K, np.float32)
    Qp = Qf @ np.asarray(Wq, np.float32).T + bq
    Kp = Kf @ np.asarray(Wk, np.float32).T + bk
    Vpp = Kf @ np.asarray(Wv, np.float32).T + bv
    Bn, Sq, _ = Qp.shape
    out = np.empty((Bn, Sq, D), np.float32)
    for b_ in range(Bn):
        for h in range(H):
            sl = slice(h * DH, (h + 1) * DH)
            s = Qp[b_][:, sl] @ Kp[b_][:, sl].T * SCALE
            s -= s.max(-1, keepdims=True)
            e = np.exp(s)
            a = e / e.sum(-1, keepdims=True)
            out[b_][:, sl] = Qp[b_][:, sl] + a @ Vpp[b_][:, sl]
    o = ln(out, g0, beta0)
    o = o + np.maximum(o @ np.asarray(Wo, np.float32).T + bo, 0.0)
    return ln(o, g1, beta1)


def kernel(**inputs):
    if not _gb_trivial(inputs["g0"], inputs["beta0"], inputs["g1"], inputs["beta1"]):
        return _kernel_numpy(**inputs)
    nc = build_nc()
    in_maps = shard_inputs(**inputs)
    res = run_bass_kernel_spmd(nc, in_maps, core_ids=list(range(NCORES)))
    out = np.empty((B, SQ_FULL, D), np.float32)
    for c in range(NCORES):
        b, half = c // QSPLIT, c % QSPLIT
        out[b, half * SQ:(half + 1) * SQ, :] = res.results[c]["out"]
    return out
